# revision 31
# baseline (speedup 1.0000x reference)
"""Trainium2 Bass kernel for nn_CustomLoss_6330781795106.

Math (derived from the reference):
  p = softmax(y_pred, axis=1) clipped to [1e-7, 1]; th = 1/C
  per row i (label l_i, argmax a_i):
    py_i  = clip(exp(y[i,l_i]) / s_i, 1e-7, 1),  s_i = sum_j exp(y[i,j])
    nl_i  = (py_i - 1) * log(clip(1 - py_i, 1e-7, 1))
    ce2_i = a_i * log(py_i)
    mask_i = [second_largest(p_i) < th]       (== cnt_i < 2; max(p_i) >= th always)
    pyD_i = mask_i * max(p_i)
  loss = sum(nl)/B + 0.01 * ( -prod(1 + pyD^2) * sum(ce2) )

Data-parallel over 8 cores (1024 rows each). Per core, the [1024, 4096] scan
produces five per-row scalars: exp-sum s (ACT Exp in-place with fused accum),
exp(max) and exp(2nd max) and argmax index (DVE max8 + max_index on the exp'd
tile; exp is strictly monotone so order/indices match y's), and the label
logit (one indirect-DMA gather). A tiny [128, 8] epilogue reduces to
per-partition partial sums; the host sums those and combines the 8 cores.

Raw Bass (no Tile): this walrus build encodes at most ONE sync-wait per
instruction, so cross-engine deps are expressed as standalone wait_ge
sequencer ops with a static 4-semaphore schedule:
  SP:   4 double-tile loads (no waits) ... out store at the end
  ACT:  8 in-place Exp+accum (each waits its load), then epilogue Ln/Exp
  DVE:  8 max8/max_index + stat copies (each waits its Exp), then epilogue
  POOL: offs load + label gather (SWDGE)
"""

import numpy as np

try:
    import concourse.bass as bass
except ImportError:  # pragma: no cover
    import sys

    sys.path.insert(0, "/opt/trn_rl_repo")
    import concourse.bass as bass

import concourse.mybir as mybir
from concourse.bass_utils import run_bass_kernel_spmd

B = 8192          # global batch
C = 4096          # classes
NCORES = 8
R = B // NCORES   # rows per core (1024)
P = 128           # partitions
T = R // P        # row-tiles per core (8)
TH = 1.0 / C
F32 = mybir.dt.float32
U32 = mybir.dt.uint32
AF = mybir.ActivationFunctionType
ALU = mybir.AluOpType
X = mybir.AxisListType.X


def _build(R=R, debug_out=False):
    T = R // P
    nc = bass.Bass("TRN2", debug=False)
    y_d = nc.dram_tensor("y", [R, C], F32, kind="ExternalInput")
    off_d = nc.dram_tensor("off", [P, T], U32, kind="ExternalInput")
    out_d = nc.dram_tensor("out", [P, 4], F32, kind="ExternalOutput")
    dbg_d = None
    if debug_out:
        dbg_d = nc.dram_tensor("dbg", [P, 8 * T], F32, kind="ExternalOutput")

    from contextlib import ExitStack
    with ExitStack() as ctx:
        def sb(name, shape, dt=F32):
            return ctx.enter_context(nc.sbuf_tensor(name, shape, dt))

        yt = sb("yt", [P, T * C])    # whole shard resident: 128 KiB/partition
        s8 = sb("s8", [P, T]); a8 = sb("a8", [P, T]); ylab = sb("ylab", [P, T])
        offs = sb("offs", [P, T], U32)
        mx8 = sb("mx8", [P, 8 * T]); ix8 = sb("ix8", [P, 8 * T], U32)
        u = sb("u", [P, T]); rs = sb("rs", [P, T]); pyr = sb("pyr", [P, T])
        py = sb("py", [P, T]); t1 = sb("t1", [P, T]); t1c = sb("t1c", [P, T])
        l1 = sb("l1", [P, T]); nlp = sb("nlp", [P, T]); lp = sb("lp", [P, T])
        ce2 = sb("ce2", [P, T]); q2 = sb("q2", [P, T]); msk = sb("msk", [P, T])
        mp = sb("mp", [P, T]); pyD = sb("pyD", [P, T]); sq = sb("sq", [P, T])
        lw = sb("lw", [P, T])
        outsb = sb("outsb", [P, 4])
        # strided [P, T] views of the per-tile top-8 stats (step 8)
        m8 = mx8[:].rearrange("p (t e) -> p t e", e=8)[:, :, 0]  # exp(max)
        v8 = mx8[:].rearrange("p (t e) -> p t e", e=8)[:, :, 1]  # exp(2nd max)
        a8u = ix8[:].rearrange("p (t e) -> p t e", e=8)[:, :, 0]  # argmax (u32)

        sem_y = ctx.enter_context(nc.semaphore("sem_y"))      # HWDGE out store
        # one sem per tile load: the 8 HWDGE queues complete out of order,
        # so a single counting sem cannot tell WHICH tiles have landed
        sem_t = [ctx.enter_context(nc.semaphore(f"sem_t{t}")) for t in range(T)]
        sem_sw = ctx.enter_context(nc.semaphore("sem_sw"))    # SWDGE offs+gather
        sem_act = ctx.enter_context(nc.semaphore("sem_act"))  # ACT progress
        sem_dve = ctx.enter_context(nc.semaphore("sem_dve"))  # DVE progress
        block = ctx.enter_context(nc.Block())

        # static DVE-op counts (sem_dve values) for cross-engine waits
        N_T1C = 2 * T + 6    # ...through t1c
        N_SQ = 2 * T + 11    # ...through sq
        N_DVE_OPS = 2 * T + 17
        # ACT-op counts (sem_act values)
        A_EXP = T            # all row exps
        A_U = T + 1          # u = exp(ylab)
        A_LP = T + 3         # l1, lp
        A_LW = T + 4         # lw

        @block.sync
        def _(sp):
            # one DMA per row-tile: the 8 HWDGE queues drain round-robin, so
            # tile t completes ~(t+1)/8 of the way through the total load
            # time and the ACT/DVE pipeline ramps up almost immediately
            for t in range(T):
                sp.dma_start(yt[:, t * C:(t + 1) * C],
                             y_d[t * P:(t + 1) * P, :]).then_inc(sem_t[t], 16)
            sp.wait_ge(sem_dve, N_DVE_OPS)
            sp.dma_start(out_d[:, :], outsb[:]).then_inc(sem_y, 16)
            ndma = 1
            if dbg_d is not None:
                dbg_groups = [s8[:], m8, v8, a8[:], ylab[:], u[:], py[:],
                              lw[:]]
                with nc.allow_non_contiguous_dma(reason="debug-only dumps"):
                    for gi, g in enumerate(dbg_groups):
                        sp.dma_start(dbg_d[:, gi * T:(gi + 1) * T],
                                     g).then_inc(sem_y, 16)
                ndma += 8
            sp.wait_ge(sem_y, 16 * ndma)  # drain the stores

        @block.gpsimd
        def _(pl):
            pl.dma_start(offs[:], off_d[:, :]).then_inc(sem_sw, 16)
            pl.wait_ge(sem_sw, 16)
            # flat element indices into [R, C] (axis=1 => coefficient 1)
            pl.indirect_dma_start(
                out=ylab[:], out_offset=None,
                in_=y_d[:, :],
                in_offset=bass.IndirectOffsetOnAxis(ap=offs[:], axis=1),
            ).then_inc(sem_sw, 16)

        @block.scalar
        def _(act):
            # in-place exp of each row-tile + fused row sum. No
            # max-subtraction: logits are standard normals, exp() is safe in
            # f32 and softmax is shift-invariant.
            for t in range(T):
                act.wait_ge(sem_t[t], 16)
                act.activation(out=yt[:, t * C:(t + 1) * C],
                               in_=yt[:, t * C:(t + 1) * C], func=AF.Exp,
                               accum_out=s8[:, t:t + 1]).then_inc(sem_act, 1)
            act.wait_ge(sem_sw, 32)
            act.activation(out=u[:], in_=ylab[:],
                           func=AF.Exp).then_inc(sem_act, 1)        # A_U
            act.wait_ge(sem_dve, N_T1C)
            act.activation(out=l1[:], in_=t1c[:],
                           func=AF.Ln).then_inc(sem_act, 1)
            act.activation(out=lp[:], in_=py[:],
                           func=AF.Ln).then_inc(sem_act, 1)         # A_LP
            act.wait_ge(sem_dve, N_SQ)
            act.activation(out=lw[:], in_=sq[:], func=AF.Ln,
                           bias=1.0).then_inc(sem_act, 1)           # A_LW

        @block.vector
        def _(dve):
            # The DVE pipeline is deep: same-engine RAW needs an explicit
            # self-semaphore (then_inc + wait) between dependent ops. dprog
            # tracks completed-DVE-op count; dwait() orders against all
            # prior DVE ops (the stream is essentially a dependence chain).
            state = {"n": 0}

            def step(inst):
                inst.then_inc(sem_dve, 1)
                state["n"] += 1

            def dwait():
                dve.wait_ge(sem_dve, state["n"])

            for t in range(T):
                dve.wait_ge(sem_act, t + 1)
                half = yt[:, t * C:(t + 1) * C]
                sl = slice(8 * t, 8 * (t + 1))
                step(dve.max(out=mx8[:, sl], in_=half))
                dwait()  # mx8 slice ready (same-engine RAW)
                step(dve.max_index(out=ix8[:, sl], in_max=mx8[:, sl],
                                   in_values=half))
            # epilogue: a linear dependence chain of tiny [P, T] ops
            step(dve.reciprocal(rs[:], s8[:]))
            dwait()
            step(dve.tensor_copy(a8[:], a8u.bitcast(mybir.dt.int32)))
            dve.wait_ge(sem_act, A_U)               # u = exp(ylab) ready
            dwait()
            step(dve.tensor_mul(pyr[:], u[:], rs[:]))
            dwait()
            step(dve.tensor_scalar(py[:], pyr[:], 1e-7, 1.0, op0=ALU.max,
                                   op1=ALU.min))
            dwait()
            step(dve.tensor_scalar(t1[:], py[:], -1.0, 1.0, op0=ALU.mult,
                                   op1=ALU.add))    # 1 - py
            dwait()
            step(dve.tensor_scalar_max(t1c[:], t1[:], 1e-7))
            step(dve.tensor_mul(q2[:], v8, rs[:]))  # second-largest prob
            dwait()
            step(dve.tensor_scalar(msk[:], q2[:], TH, None, op0=ALU.is_lt))
            step(dve.tensor_mul(mp[:], m8, rs[:]))  # max prob
            dwait()
            step(dve.tensor_mul(pyD[:], msk[:], mp[:]))
            dwait()
            step(dve.tensor_mul(sq[:], pyD[:], pyD[:]))
            dve.wait_ge(sem_act, A_LP)              # l1, lp ready
            step(dve.tensor_mul(nlp[:], t1[:], l1[:]))  # host negates
            step(dve.tensor_mul(ce2[:], a8[:], lp[:]))
            dwait()
            step(dve.tensor_reduce(outsb[:, 0:1], nlp[:], axis=X, op=ALU.add))
            step(dve.tensor_reduce(outsb[:, 1:2], ce2[:], axis=X, op=ALU.add))
            dve.wait_ge(sem_act, A_LW)              # lw ready
            step(dve.tensor_reduce(outsb[:, 2:3], lw[:], axis=X, op=ALU.add))
            dwait()
            step(dve.tensor_reduce(outsb[:, 3:4], a8[:], axis=X, op=ALU.add))
            assert state["n"] == N_DVE_OPS, state["n"]
    return nc


def _in_maps(y, lab):
    maps = []
    for c in range(NCORES):
        ys = np.ascontiguousarray(y[c * R:(c + 1) * R])
        labs = lab[c * R:(c + 1) * R].astype(np.int64)
        r = np.arange(R, dtype=np.int64)
        flat = (r * C + labs).astype(np.uint32)
        off = np.ascontiguousarray(flat.reshape(T, P).T)  # [P, T]
        maps.append({"y": ys, "off": off})
    return maps


def _combine(results):
    nlp_sum = 0.0
    ce2_sum = 0.0
    lw_sum = 0.0
    for c in range(NCORES):
        o = np.asarray(results[c]["out"], dtype=np.float64)
        nlp_sum += o[:, 0].sum()
        ce2_sum += o[:, 1].sum()
        lw_sum += o[:, 2].sum()
    nl = -nlp_sum / float(B)
    pl = -np.exp(lw_sum) * ce2_sum
    return np.array([nl + 0.01 * pl], dtype=np.float32)


def kernel(y_pred, y_true2):
    y = np.ascontiguousarray(np.asarray(y_pred, dtype=np.float32))
    lab = np.asarray(y_true2).astype(np.int64)
    assert y.shape == (B, C) and lab.shape == (B,)
    nc = _build()
    res = run_bass_kernel_spmd(nc, _in_maps(y, lab),
                               core_ids=list(range(NCORES))).results
    return _combine(res)
